# revision 51
# baseline (speedup 1.0000x reference)
"""Trainium2 Bass kernel for the colorization loss — v5.

Math (validated to rel-err ~1e-6):
  m(q)  = 2*a*gx_q + 2*b*gy_q - |g_q|^2        # = (a^2+b^2) - d^2(q)
  top-5 largest m == 5 nearest bins (sorted);  e_k = exp((m_k - m_0)/50)
  loss  = mean( (lse * sum_k(reb_k e_k) - sum_k(reb_k e_k zbar_k)) / sum_k e_k )
  with lse = log(sum_q exp(zbar_q)).

v5 = v2 + per-tile gamut candidate pruning:
  Pixels are HOST-SORTED in Morton order over (a,b), so each 128-pixel
  tile occupies a small patch of ab-space.  For each tile the host
  computes (from the tiny gamut + the tile's bbox, rigorously via the
  d5 Lipschitz bound) the set of bins that can possibly be in any of
  its pixels' top-5 — mean ~40 of 313 (verified exact on all pixels).
  Each tile's matmul then only produces its own candidate columns and
  DVE's top-8 scan shrinks from 313 to K_t columns (451ns -> ~170ns
  avg), moving the bottleneck to ACT's exp stream (~74us floor).
  Per-pixel loss partials are order-independent, so the sort needs no
  undo.  The exp/sum(exp) path is unchanged except the add-tree is
  rebalanced: level 1 on Pool, the rest + final reduce on the
  now-lighter DVE.  3 abx segments (bases 0/32/64), each laid out
  [per-tile candidate blocks | a;b;1 pixel columns].
"""

import numpy as np
import ml_dtypes

import concourse.bass as bass
import concourse.tile as tile
from concourse import mybir
from concourse.bass_utils import run_bass_kernel_spmd

# Problem shape (hardcoded: nn_ColorizationLoss, B,H,W,Q = 16,128,128,313)
B, H, W, Q = 16, 128, 128, 313
NCORES = 8
B_PER = B // NCORES            # 2 images per core
PIX = B_PER * H * W            # 32768 pixels per core
P = 128                        # SBUF partitions / pixels per tile
NT = PIX // P                  # 256 tiles per core
GT = 16                        # tiles per zbar group
NG = NT // GT                  # 16 groups
TB = 64                        # tiles per epilogue batch
NB = NT // TB                  # 8 batches
TOPK = 5
INV50 = 1.0 / 50.0             # 1/(2*sigma^2), sigma=5
QPAD = 320                     # es padded to 320 cols for a clean add-tree
ESBUF = 5                      # rotating es buffers

NSEG = 3                       # abx partition segments (bases 0/32/64)
SEG_TILES = (86, 85, 85)
SEG_START = (0, 86, 171)

f32 = mybir.dt.float32
bf16 = mybir.dt.bfloat16
f32r = mybir.dt.float32r
AF = mybir.ActivationFunctionType
AX = mybir.AxisListType
ALU = mybir.AluOpType

_NC = None
_NC_KEY = None


def _build_nc(Kp, goff, segc):
    """Kp[t]: padded candidate count per tile; goff[t]: column offset of
    tile t's gamut block within its segment; segc: columns per abx row."""
    nc = bass.Bass()
    zbar_d = nc.dram_tensor("zbar", [PIX, Q], bf16, kind="ExternalInput")
    abx_d = nc.dram_tensor("abx", [72, segc], bf16, kind="ExternalInput")
    zf_d = nc.dram_tensor("zf", [P, NT * TOPK], bf16, kind="ExternalInput")
    reb_d = nc.dram_tensor("rebt", [P, TB * TOPK], bf16, kind="ExternalInput")
    out_d = nc.dram_tensor("acc", [P, NT], f32, kind="ExternalOutput")

    # pixel-column start per segment (gamut blocks lead the segment row)
    gtot = [goff[SEG_START[s]] + sum(Kp[t] for t in range(SEG_START[s],
            SEG_START[s] + SEG_TILES[s])) - goff[SEG_START[s]]
            for s in range(NSEG)]
    garea = [sum(Kp[SEG_START[s]:SEG_START[s] + SEG_TILES[s]])
             for s in range(NSEG)]
    pix0 = [garea[s] for s in range(NSEG)]

    zbar_g = zbar_d[:, :].rearrange("(g j p) q -> g p j q", j=GT, p=P)
    zbar_g2 = zbar_d[:, :].rearrange("(g j p) q -> g p j q", j=2 * GT, p=P)

    with tile.TileContext(nc) as tc:
        with (
            tc.tile_pool(name="singles", bufs=1) as singles,
            tc.tile_pool(name="zg", bufs=3) as zgp,
            tc.tile_pool(name="tree", bufs=2) as trp,
            tc.tile_pool(name="epi", bufs=1) as epi_big,
            tc.tile_pool(name="epi2", bufs=2) as epi_small,
            tc.tile_pool(name="ps", bufs=4, space="PSUM") as psp,
            tc.tile_pool(name="psb", bufs=1, space="PSUM") as psbp,
        ):
            abx_sb = singles.tile([8 + 32 * (NSEG - 1), segc], bf16)
            # one [67, cols] dma moves the same column range of all three
            # segments in parallel partition lanes (the DMA model charges
            # max bytes PER PARTITION, so narrow 3-row transfers are 3x
            # the cost of this stacked layout).  gamut blocks ride the
            # gpsimd ring; pixel columns stream on the scalar ring.
            GAR = max(garea)
            # es pads zeroed FIRST (decode before the dma SEQ-holds; the
            # DVE-run early sum trees need them)
            # paired 32-tile buffers: each sync dma pays ~2.2us fixed
            # overhead, so 8 paired loads beat 16 single-group loads
            es_bufs = []
            for i in range(3):
                e = singles.tile([P, 2 * GT, QPAD], bf16, name=f"es{i}")
                nc.gpsimd.memset(e[:, :, Q:QPAD], 0.0)
                es_bufs.append(e)
            nc.gpsimd.dma_start(out=abx_sb[0:72, 0:GAR], in_=abx_d[:, 0:GAR])

            zf_sb = singles.tile([P, NT, TOPK], bf16)
            nc.gpsimd.dma_start(
                out=zf_sb, in_=zf_d[:, :].rearrange("p (t k) -> p t k", k=TOPK)
            )
            reb_sb = singles.tile([P, TB, TOPK], bf16)
            nc.gpsimd.dma_start(
                out=reb_sb, in_=reb_d[:, :].rearrange("p (t k) -> p t k", k=TOPK)
            )
            acc = singles.tile([P, NT], f32)

            Sf = singles.tile([P, NT], f32)          # sum_q exp(zbar)
            Wf = singles.tile([P, NT, 8], f32)       # top-8 of m

            def issue_pair(p):
                zgt = es_bufs[p % 3]
                nc.sync.dma_start(out=zgt[:, :, 0:Q], in_=zbar_g2[p])

            # pair0 via two 16-tile dmas (short head), pairs after
            nc.sync.dma_start(out=es_bufs[0][:, 0:GT, 0:Q], in_=zbar_g[0])
            nc.sync.dma_start(out=es_bufs[0][:, GT:2 * GT, 0:Q], in_=zbar_g[1])
            # exp(0) decodes BEFORE the pixel pieces: engine ops release
            # the ACT SEQ at dispatch, so the dma SEQ-holds below hide
            # under exp(0)'s engine time instead of delaying the spine
            e0 = es_bufs[0]
            nc.scalar.activation(out=e0[:, 0:GT, 0:Q], in_=e0[:, 0:GT, 0:Q],
                                 func=AF.Exp)
            # pixel columns in 3 pieces on the scalar ring, column order
            PXP = [GAR, GAR + 32 * P, GAR + 64 * P, segc]
            for pi in range(3):
                nc.scalar.dma_start(out=abx_sb[0:72, PXP[pi]:PXP[pi + 1]],
                                    in_=abx_d[:, PXP[pi]:PXP[pi + 1]])
            issue_pair(1)

            for g in range(NG):
                if g % 2 == 0 and g // 2 + 2 < NG // 2:
                    issue_pair(g // 2 + 2)
                eb = (g % 2) * GT

                # PE: 16 matmuls on per-tile candidate blocks; DVE max8
                for j in range(GT):
                    t = g * GT + j
                    seg = 0 if t < SEG_START[1] else (1 if t < SEG_START[2] else 2)
                    so = 32 * seg
                    col = max(garea) + (t - SEG_START[seg]) * P
                    k = Kp[t]
                    # fresh psum slot where the matmul's abx piece may be
                    # the last-arriving wait (single-wait LDWEIGHTS rule)
                    if t in (0, 16, SEG_START[1], SEG_START[2]):
                        ps = psbp.tile([P, QPAD], f32, tag=f"psb{t}", name=f"psb{t}")
                    else:
                        ps = psp.tile([P, QPAD], f32, tag="ps")
                    nc.tensor.matmul(
                        ps[:, 0:k],
                        abx_sb[so:so + 8, col:col + P],
                        abx_sb[so:so + 8, goff[t]:goff[t] + k],
                        start=True,
                        stop=True,
                    )
                    nc.vector.max(out=Wf[:, t, :], in_=ps[:, 0:k])

                # ACT: one batched exp per group, IN PLACE (g0 split so
                # the first exp starts on the first dma piece; last group
                # split 24/8 so the tail tree+epilogue overlap exp)
                es = es_bufs[(g >> 1) % 3]
                ze = es[:, eb:eb + GT, 0:Q]
                if g == 0:
                    pass  # exp(0) emitted before the pixel pieces above
                elif g == NG - 1:
                    nc.scalar.activation(out=es[:, eb:eb + 8, 0:Q],
                                         in_=es[:, eb:eb + 8, 0:Q], func=AF.Exp)
                    nc.scalar.activation(out=es[:, eb + 8:eb + GT, 0:Q],
                                         in_=es[:, eb + 8:eb + GT, 0:Q], func=AF.Exp)
                else:
                    nc.scalar.activation(out=ze, in_=ze, func=AF.Exp)

                # sum tree: level 1 on Pool, rest + reduce on DVE
                def emit_tree(j0, w, l1_eng, sfx2=""):
                    # full-width tiles (shared tags) sliced to w: the half
                    # trees of the last group reuse the main tree buffers
                    sl2 = slice(g * GT + j0, g * GT + j0 + w)
                    t160f = trp.tile([P, GT, 160], bf16, tag="t160",
                                     name=f"t160_{g}{sfx2}")
                    t160 = t160f[:, 0:w, :]
                    l1_eng.tensor_add(t160, es[:, eb + j0:eb + j0 + w, 0:160],
                                      es[:, eb + j0:eb + j0 + w, 160:320])
                    t80f = trp.tile([P, GT, 80], bf16, tag="t80",
                                    name=f"t80_{g}{sfx2}")
                    t80 = t80f[:, 0:w, :]
                    nc.vector.tensor_add(t80, t160[:, :, 0:80],
                                         t160[:, :, 80:160])
                    t40f = trp.tile([P, GT, 40], bf16, tag="t40",
                                    name=f"t40_{g}{sfx2}")
                    t40 = t40f[:, 0:w, :]
                    nc.vector.tensor_add(t40, t80[:, :, 0:40], t80[:, :, 40:80])
                    nc.vector.tensor_reduce(
                        Sf[:, sl2].rearrange("p (t one) -> p t one", one=1),
                        t40, AX.X, ALU.add,
                    )

                if g < NG - 1:
                    emit_tree(0, GT, nc.gpsimd)

                # ---- epilogue (from v2): batches 0-6 of 32 tiles; the last
                # 32 tiles split 16/15/1 so the tail chain stays short ----
                def emit_epi(sl, col, wb, sfx, raw=False):
                    epi = epi_small if wb <= 8 else epi_big
                    sub_eng = nc.vector if raw else nc.gpsimd
                    te = nc.gpsimd
                    Xb = epi.tile([P, wb, TOPK], bf16, tag=f"X{wb}", name=f"X{sfx}")
                    sub_eng.tensor_sub(
                        Xb, Wf[:, sl, 0:TOPK],
                        Wf[:, sl, 0:1].broadcast_to([P, wb, TOPK]),
                    )
                    E = epi.tile([P, wb, TOPK], bf16, tag=f"E{wb}", name=f"E{sfx}")
                    nc.scalar.activation(out=E, in_=Xb, func=AF.Exp,
                                         scale=INV50)
                    lse = epi.tile([P, wb], f32, tag=f"lse{wb}", name=f"lse{sfx}")
                    nc.scalar.activation(out=lse, in_=Sf[:, sl], func=AF.Ln)

                    def pool_sum5(nm, x):
                        y2 = epi.tile([P, wb, 2], f32, tag=f"{nm}2{sfx}", name=f"{nm}2{sfx}")
                        te.tensor_add(y2, x[:, :, 0:2], x[:, :, 2:4])
                        y1 = epi.tile([P, wb, 1], f32, tag=f"{nm}1{sfx}", name=f"{nm}1{sfx}")
                        te.tensor_add(y1, y2[:, :, 0:1], y2[:, :, 1:2])
                        y0 = epi.tile([P, wb], f32, tag=f"{nm}0{sfx}", name=f"{nm}0{sfx}")
                        te.tensor_add(
                            y0.rearrange("p (t one) -> p t one", one=1),
                            y1, x[:, :, 4:5])
                        return y0

                    sw = pool_sum5("sw", E)
                    r = epi.tile([P, wb], f32, tag=f"r{wb}", name=f"r{sfx}")
                    if raw:
                        nc.vector.reciprocal(r, sw)
                    else:
                        nlsw = epi.tile([P, wb], f32, tag=f"nlsw{wb}", name=f"nlsw{sfx}")
                        nc.scalar.activation(out=nlsw, in_=sw, func=AF.Ln)
                        nc.scalar.activation(out=r, in_=nlsw, func=AF.Exp,
                                             scale=-1.0)
                    U = epi.tile([P, wb, TOPK], bf16, tag=f"U{wb}", name=f"U{sfx}")
                    te.tensor_mul(U, E, reb_sb[:, 0:wb])
                    UZ = epi.tile([P, wb, TOPK], bf16, tag=f"UZ{wb}", name=f"UZ{sfx}")
                    te.tensor_mul(UZ, U, zf_sb[:, sl])
                    s1 = pool_sum5("s1", UZ)
                    s2 = pool_sum5("s2", U)
                    t1 = epi.tile([P, wb], f32, tag=f"t1{wb}", name=f"t1{sfx}")
                    te.tensor_mul(t1, lse, s2)
                    t1b = epi.tile([P, wb], f32, tag=f"t1b{wb}", name=f"t1b{sfx}")
                    te.tensor_sub(t1b, t1, s1)
                    if raw:
                        te.tensor_mul(acc[:, col:col + wb], t1b, r)
                        return
                    nc.gpsimd.tensor_mul(t1b, t1b, r)
                    pp = t1b
                    w2 = wb // 2
                    lv = 0
                    while w2 >= 1:
                        nxt = epi.tile([P, w2], f32, tag=f"pp{lv}{sfx}", name=f"pp{lv}{sfx}")
                        nc.gpsimd.tensor_add(nxt, pp[:, 0:w2], pp[:, w2:2 * w2])
                        pp = nxt
                        w2 //= 2
                        lv += 1
                    nc.gpsimd.tensor_copy(acc[:, col:col + 1], pp)

                if g in (3, 7, 11):
                    bi = g // 4
                    emit_epi(slice(bi * TB, (bi + 1) * TB), bi * TB, TB,
                             f"b{bi}", raw=True)
                elif g == NG - 2:
                    emit_epi(slice(192, 240), 192, 48, "h", raw=True)
                elif g == NG - 1:
                    emit_tree(0, 8, nc.vector, "a")
                    emit_epi(slice(240, 248), 240, 8, "h2a", raw=True)
                    emit_tree(8, 8, nc.vector, "b")
                    emit_epi(slice(248, 256), 248, 8, "h2b", raw=True)

            nc.sync.dma_start(out=out_d[:, :], in_=acc)

    # tail drain: keep only the out-DMA completion wait (see v2)
    out_sems = set()
    for blk in nc.m.functions[0].blocks:
        for inst in blk.instructions:
            si = getattr(inst, "sync_info", None)
            if si is None or type(inst).__name__ != "InstDMACopy":
                continue
            try:
                if inst.outs[0].memref == "acc":
                    out_sems |= {u.ant_name for u in si.on_update}
            except Exception:
                pass
    assert out_sems, "could not locate the output DMA's completion sem"
    for blk in nc.m.functions[0].blocks:
        for inst in blk.instructions:
            si = getattr(inst, "sync_info", None)
            if si is None or type(inst).__name__ != "InstDrain":
                continue
            ge = [w for w in si.on_wait if w.wait_mode == "sem-ge-imm"]
            if len(ge) >= 2:
                keep = [w for w in ge if w.ant_name in out_sems]
                assert keep, f"tail drain has no out-DMA wait: {ge}"
                si.on_wait = keep[:1]
    import bass_rust
    bass_rust.generate_event_semaphores(nc)
    return nc


def _get_nc(Kp=None, goff=None, segc=None):
    global _NC, _NC_KEY
    if Kp is None:
        assert _NC is not None, "kernel not built yet"
        return _NC
    key = (tuple(Kp), segc)
    if _NC is None or _NC_KEY != key:
        _NC = _build_nc(Kp, goff, segc)
        _NC_KEY = key
    return _NC


def _morton(a, b, bits=8):
    qa = np.clip(((a + 110.0) / 220.0 * (1 << bits)).astype(np.int64),
                 0, (1 << bits) - 1)
    qb = np.clip(((b + 110.0) / 220.0 * (1 << bits)).astype(np.int64),
                 0, (1 << bits) - 1)
    m = np.zeros_like(qa)
    for i in range(bits):
        m |= ((qa >> i) & 1) << (2 * i) | ((qb >> i) & 1) << (2 * i + 1)
    return m


def make_in_maps(Zbar, Y, rebalance, gamut):
    Zbar = np.asarray(Zbar, dtype=np.float32)
    Y = np.asarray(Y, dtype=np.float32)
    rebalance = np.asarray(rebalance, dtype=np.float32)
    gamut = np.asarray(gamut, dtype=np.float32)
    gx, gy = gamut[:, 0], gamut[:, 1]
    gn = gx * gx + gy * gy

    rebt = np.ascontiguousarray(
        np.broadcast_to(np.tile(rebalance[:TOPK], TB)[None, :], (P, TB * TOPK))
    ).astype(ml_dtypes.bfloat16)

    # --- per-core: Morton-sort pixels, build per-tile candidate sets ---
    per_core = []
    for c in range(NCORES):
        sl = slice(c * B_PER, (c + 1) * B_PER)
        a = Y[sl, 1].reshape(PIX)
        b = Y[sl, 2].reshape(PIX)
        order = np.argsort(_morton(a, b), kind="stable")
        a, b = a[order], b[order]
        cands = []
        for t in range(NT):
            ts = slice(t * P, (t + 1) * P)
            ta, tb = a[ts], b[ts]
            lo = np.array([ta.min(), tb.min()])
            hi = np.array([ta.max(), tb.max()])
            ctr = (lo + hi) / 2
            halfdiag = float(np.linalg.norm(hi - lo)) / 2
            dc = np.sqrt(((gamut - ctr) ** 2).sum(1))
            d5c = np.partition(dc, TOPK - 1)[TOPK - 1]
            dx = np.maximum(np.maximum(lo[0] - gx, gx - hi[0]), 0)
            dy = np.maximum(np.maximum(lo[1] - gy, gy - hi[1]), 0)
            dbb = np.sqrt(dx * dx + dy * dy)
            cand = np.where(dbb <= d5c + halfdiag + 1e-3)[0]
            cands.append(cand)
        per_core.append((order, a, b, cands))

    # shared padded widths across cores (one kernel build): per tile index,
    # max over cores, padded to a multiple of 8, min 16
    Kp = []
    for t in range(NT):
        k = max(len(per_core[c][3][t]) for c in range(NCORES))
        Kp.append(max((k + 7) // 8 * 8, 16))
    goff = []
    off = 0
    seg_of = lambda t: 0 if t < SEG_START[1] else (1 if t < SEG_START[2] else 2)
    for t in range(NT):
        if t in SEG_START:
            off = 0
        goff.append(off)
        off += Kp[t]
    garea = [sum(Kp[SEG_START[s]:SEG_START[s] + SEG_TILES[s]])
             for s in range(NSEG)]
    segc = max(garea) + max(SEG_TILES) * P

    in_maps = []
    for c in range(NCORES):
        order, a, b, cands = per_core[c]
        sl = slice(c * B_PER, (c + 1) * B_PER)
        zb = Zbar[sl].reshape(PIX, Q)[order]
        zb16 = np.ascontiguousarray(zb.astype(ml_dtypes.bfloat16))
        zf = np.ascontiguousarray(
            zb[:, 0:TOPK].reshape(NT, P, TOPK).transpose(1, 0, 2).reshape(P, NT * TOPK)
        ).astype(ml_dtypes.bfloat16)
        # bf16 hi/lo split: m = ah*gxh + ah*gxl + al*gxh + (same for b/gy)
        #                     + 1*(-nh) + 1*(-nl)   (8-row contraction)
        def hl(x):
            h = x.astype(ml_dtypes.bfloat16)
            l = (x - h.astype(np.float32)).astype(ml_dtypes.bfloat16)
            return h, l
        abx = np.zeros((72, segc), ml_dtypes.bfloat16)
        GAR = max(garea)
        gxh, gxl = hl(2.0 * gx)
        gyh, gyl = hl(2.0 * gy)
        gnh, gnl = hl(-gn)
        ah, al = hl(a)
        bh, bl = hl(b)
        for s in range(NSEG):
            r = 32 * s
            npx = SEG_TILES[s] * P
            px = slice(SEG_START[s] * P, SEG_START[s] * P + npx)
            p0 = GAR
            for ri, row in enumerate((ah[px], ah[px], al[px],
                                      bh[px], bh[px], bl[px])):
                abx[r + ri, p0:p0 + npx] = row
            abx[r + 6, p0:p0 + npx] = 1.0
            abx[r + 7, p0:p0 + npx] = 1.0
            for t in range(SEG_START[s], SEG_START[s] + SEG_TILES[s]):
                cd = cands[t]
                o = goff[t]
                n = len(cd)
                for ri, row in enumerate((gxh[cd], gxl[cd], gxh[cd],
                                          gyh[cd], gyl[cd], gyh[cd],
                                          gnh[cd], gnl[cd])):
                    abx[r + ri, o:o + n] = row
                abx[r + 6, o + n:o + Kp[t]] = -3.0e38
                abx[r + 7, o + n:o + Kp[t]] = 0.0
        in_maps.append({"zbar": zb16, "abx": abx, "zf": zf, "rebt": rebt})
    return in_maps, Kp, goff, segc


def kernel(Zbar, Y, rebalance, gamut):
    in_maps, Kp, goff, segc = make_in_maps(Zbar, Y, rebalance, gamut)
    nc = _get_nc(Kp, goff, segc)
    res = run_bass_kernel_spmd(nc, in_maps, list(range(NCORES)))
    total = sum(float(r["acc"].sum(dtype=np.float64)) for r in res.results)
    return np.float32(total / (B * H * W))


# revision 52
# speedup vs baseline: 1.0004x; 1.0004x over previous
"""Trainium2 Bass kernel for the colorization loss — v5.

Math (validated to rel-err ~1e-6):
  m(q)  = 2*a*gx_q + 2*b*gy_q - |g_q|^2        # = (a^2+b^2) - d^2(q)
  top-5 largest m == 5 nearest bins (sorted);  e_k = exp((m_k - m_0)/50)
  loss  = mean( (lse * sum_k(reb_k e_k) - sum_k(reb_k e_k zbar_k)) / sum_k e_k )
  with lse = log(sum_q exp(zbar_q)).

v5 = v2 + per-tile gamut candidate pruning:
  Pixels are HOST-SORTED in Morton order over (a,b), so each 128-pixel
  tile occupies a small patch of ab-space.  For each tile the host
  computes (from the tiny gamut + the tile's bbox, rigorously via the
  d5 Lipschitz bound) the set of bins that can possibly be in any of
  its pixels' top-5 — mean ~40 of 313 (verified exact on all pixels).
  Each tile's matmul then only produces its own candidate columns and
  DVE's top-8 scan shrinks from 313 to K_t columns (451ns -> ~170ns
  avg), moving the bottleneck to ACT's exp stream (~74us floor).
  Per-pixel loss partials are order-independent, so the sort needs no
  undo.  The exp/sum(exp) path is unchanged except the add-tree is
  rebalanced: level 1 on Pool, the rest + final reduce on the
  now-lighter DVE.  3 abx segments (bases 0/32/64), each laid out
  [per-tile candidate blocks | a;b;1 pixel columns].
"""

import numpy as np
import ml_dtypes

import concourse.bass as bass
import concourse.tile as tile
from concourse import mybir
from concourse.bass_utils import run_bass_kernel_spmd

# Problem shape (hardcoded: nn_ColorizationLoss, B,H,W,Q = 16,128,128,313)
B, H, W, Q = 16, 128, 128, 313
NCORES = 8
B_PER = B // NCORES            # 2 images per core
PIX = B_PER * H * W            # 32768 pixels per core
P = 128                        # SBUF partitions / pixels per tile
NT = PIX // P                  # 256 tiles per core
GT = 16                        # tiles per zbar group
NG = NT // GT                  # 16 groups
TB = 64                        # tiles per epilogue batch
NB = NT // TB                  # 8 batches
TOPK = 5
INV50 = 1.0 / 50.0             # 1/(2*sigma^2), sigma=5
QPAD = 320                     # es padded to 320 cols for a clean add-tree
ESBUF = 5                      # rotating es buffers

NSEG = 3                       # abx partition segments (bases 0/32/64)
SEG_TILES = (86, 85, 85)
SEG_START = (0, 86, 171)

f32 = mybir.dt.float32
bf16 = mybir.dt.bfloat16
f32r = mybir.dt.float32r
AF = mybir.ActivationFunctionType
AX = mybir.AxisListType
ALU = mybir.AluOpType

_NC = None
_NC_KEY = None


def _build_nc(Kp, goff, segc):
    """Kp[t]: padded candidate count per tile; goff[t]: column offset of
    tile t's gamut block within its segment; segc: columns per abx row."""
    nc = bass.Bass()
    zbar_d = nc.dram_tensor("zbar", [PIX, Q], bf16, kind="ExternalInput")
    abx_d = nc.dram_tensor("abx", [72, segc], bf16, kind="ExternalInput")
    zf_d = nc.dram_tensor("zf", [P, NT * TOPK], bf16, kind="ExternalInput")
    reb_d = nc.dram_tensor("rebt", [P, TB * TOPK], bf16, kind="ExternalInput")
    out_d = nc.dram_tensor("acc", [P, NT], f32, kind="ExternalOutput")

    # pixel-column start per segment (gamut blocks lead the segment row)
    gtot = [goff[SEG_START[s]] + sum(Kp[t] for t in range(SEG_START[s],
            SEG_START[s] + SEG_TILES[s])) - goff[SEG_START[s]]
            for s in range(NSEG)]
    garea = [sum(Kp[SEG_START[s]:SEG_START[s] + SEG_TILES[s]])
             for s in range(NSEG)]
    pix0 = [garea[s] for s in range(NSEG)]

    zbar_g = zbar_d[:, :].rearrange("(g j p) q -> g p j q", j=GT, p=P)

    with tile.TileContext(nc) as tc:
        with (
            tc.tile_pool(name="singles", bufs=1) as singles,
            tc.tile_pool(name="zg", bufs=3) as zgp,
            tc.tile_pool(name="tree", bufs=2) as trp,
            tc.tile_pool(name="epi", bufs=1) as epi_big,
            tc.tile_pool(name="epi2", bufs=2) as epi_small,
            tc.tile_pool(name="ps", bufs=4, space="PSUM") as psp,
            tc.tile_pool(name="psb", bufs=1, space="PSUM") as psbp,
        ):
            abx_sb = singles.tile([8 + 32 * (NSEG - 1), segc], bf16)
            # one [67, cols] dma moves the same column range of all three
            # segments in parallel partition lanes (the DMA model charges
            # max bytes PER PARTITION, so narrow 3-row transfers are 3x
            # the cost of this stacked layout).  gamut blocks ride the
            # gpsimd ring; pixel columns stream on the scalar ring.
            GAR = max(garea)
            # es pads zeroed FIRST (decode before the dma SEQ-holds; the
            # DVE-run early sum trees need them)
            es_bufs = []
            for i in range(ESBUF):
                e = singles.tile([P, GT, QPAD], bf16, name=f"es{i}")
                nc.gpsimd.memset(e[:, :, Q:QPAD], 0.0)
                es_bufs.append(e)
            nc.gpsimd.dma_start(out=abx_sb[0:72, 0:GAR], in_=abx_d[:, 0:GAR])

            zf_sb = singles.tile([P, NT, TOPK], bf16)
            nc.gpsimd.dma_start(
                out=zf_sb, in_=zf_d[:, :].rearrange("p (t k) -> p t k", k=TOPK)
            )
            reb_sb = singles.tile([P, TB, TOPK], bf16)
            nc.gpsimd.dma_start(
                out=reb_sb, in_=reb_d[:, :].rearrange("p (t k) -> p t k", k=TOPK)
            )
            acc = singles.tile([P, NT], f32)

            Sf = singles.tile([P, NT], f32)          # sum_q exp(zbar)
            Wf = singles.tile([P, NT, 8], f32)       # top-8 of m

            def issue_zg(g):
                zgt = es_bufs[g % ESBUF]
                nc.sync.dma_start(out=zgt[:, :, 0:Q], in_=zbar_g[g])

            # zg0 arrives in two pieces so exp can start ~1.5us earlier
            nc.sync.dma_start(out=es_bufs[0][:, 0:4, 0:Q], in_=zbar_g[0][:, 0:4, :])
            nc.sync.dma_start(out=es_bufs[0][:, 4:GT, 0:Q], in_=zbar_g[0][:, 4:GT, :])
            # exp(0) decodes BEFORE the pixel pieces: engine ops release
            # the ACT SEQ at dispatch, so the dma SEQ-holds below hide
            # under exp(0)'s engine time instead of delaying the spine
            e0 = es_bufs[0]
            nc.scalar.activation(out=e0[:, 0:4, 0:Q], in_=e0[:, 0:4, 0:Q],
                                 func=AF.Exp)
            nc.scalar.activation(out=e0[:, 4:GT, 0:Q], in_=e0[:, 4:GT, 0:Q],
                                 func=AF.Exp)
            # pixel columns in 3 pieces on the scalar ring, column order
            PXP = [GAR, GAR + 32 * P, GAR + 64 * P, segc]
            for pi in range(3):
                nc.scalar.dma_start(out=abx_sb[0:72, PXP[pi]:PXP[pi + 1]],
                                    in_=abx_d[:, PXP[pi]:PXP[pi + 1]])
            issue_zg(1)

            for g in range(NG):
                if g + 2 < NG:
                    issue_zg(g + 2)

                # PE: 16 matmuls on per-tile candidate blocks; DVE max8
                for j in range(GT):
                    t = g * GT + j
                    seg = 0 if t < SEG_START[1] else (1 if t < SEG_START[2] else 2)
                    so = 32 * seg
                    col = max(garea) + (t - SEG_START[seg]) * P
                    k = Kp[t]
                    # fresh psum slot where the matmul's abx piece may be
                    # the last-arriving wait (single-wait LDWEIGHTS rule)
                    if t in (0, 16, SEG_START[1], SEG_START[2]):
                        ps = psbp.tile([P, QPAD], f32, tag=f"psb{t}", name=f"psb{t}")
                    else:
                        ps = psp.tile([P, QPAD], f32, tag="ps")
                    nc.tensor.matmul(
                        ps[:, 0:k],
                        abx_sb[so:so + 8, col:col + P],
                        abx_sb[so:so + 8, goff[t]:goff[t] + k],
                        start=True,
                        stop=True,
                    )
                    nc.vector.max(out=Wf[:, t, :], in_=ps[:, 0:k])

                # ACT: one batched exp per group, IN PLACE (g0 split so
                # the first exp starts on the first dma piece; last group
                # split 24/8 so the tail tree+epilogue overlap exp)
                es = es_bufs[g % ESBUF]
                ze = es[:, :, 0:Q]
                if g == 0:
                    pass  # exp(0) emitted before the pixel pieces above
                elif g == NG - 1:
                    nc.scalar.activation(out=es[:, 0:8, 0:Q], in_=es[:, 0:8, 0:Q],
                                         func=AF.Exp)
                    nc.scalar.activation(out=es[:, 8:GT, 0:Q], in_=es[:, 8:GT, 0:Q],
                                         func=AF.Exp)
                else:
                    nc.scalar.activation(out=ze, in_=ze, func=AF.Exp)

                # sum tree: level 1 on Pool, rest + reduce on DVE
                def emit_tree(j0, w, l1_eng, sfx2=""):
                    # full-width tiles (shared tags) sliced to w: the half
                    # trees of the last group reuse the main tree buffers
                    sl2 = slice(g * GT + j0, g * GT + j0 + w)
                    t160f = trp.tile([P, GT, 160], bf16, tag="t160",
                                     name=f"t160_{g}{sfx2}")
                    t160 = t160f[:, 0:w, :]
                    l1_eng.tensor_add(t160, es[:, j0:j0 + w, 0:160],
                                      es[:, j0:j0 + w, 160:320])
                    t80f = trp.tile([P, GT, 80], bf16, tag="t80",
                                    name=f"t80_{g}{sfx2}")
                    t80 = t80f[:, 0:w, :]
                    nc.vector.tensor_add(t80, t160[:, :, 0:80],
                                         t160[:, :, 80:160])
                    t40f = trp.tile([P, GT, 40], bf16, tag="t40",
                                    name=f"t40_{g}{sfx2}")
                    t40 = t40f[:, 0:w, :]
                    nc.vector.tensor_add(t40, t80[:, :, 0:40], t80[:, :, 40:80])
                    nc.vector.tensor_reduce(
                        Sf[:, sl2].rearrange("p (t one) -> p t one", one=1),
                        t40, AX.X, ALU.add,
                    )

                if g < NG - 1:
                    emit_tree(0, GT, nc.gpsimd)

                # ---- epilogue (from v2): batches 0-6 of 32 tiles; the last
                # 32 tiles split 16/15/1 so the tail chain stays short ----
                def emit_epi(sl, col, wb, sfx, raw=False):
                    epi = epi_small if wb <= 8 else epi_big
                    sub_eng = nc.vector if raw else nc.gpsimd
                    te = nc.gpsimd
                    Xb = epi.tile([P, wb, TOPK], bf16, tag=f"X{wb}", name=f"X{sfx}")
                    sub_eng.tensor_sub(
                        Xb, Wf[:, sl, 0:TOPK],
                        Wf[:, sl, 0:1].broadcast_to([P, wb, TOPK]),
                    )
                    E = epi.tile([P, wb, TOPK], bf16, tag=f"E{wb}", name=f"E{sfx}")
                    nc.scalar.activation(out=E, in_=Xb, func=AF.Exp,
                                         scale=INV50)
                    lse = epi.tile([P, wb], f32, tag=f"lse{wb}", name=f"lse{sfx}")
                    nc.scalar.activation(out=lse, in_=Sf[:, sl], func=AF.Ln)

                    def pool_sum5(nm, x):
                        y2 = epi.tile([P, wb, 2], f32, tag=f"{nm}2{sfx}", name=f"{nm}2{sfx}")
                        te.tensor_add(y2, x[:, :, 0:2], x[:, :, 2:4])
                        y1 = epi.tile([P, wb, 1], f32, tag=f"{nm}1{sfx}", name=f"{nm}1{sfx}")
                        te.tensor_add(y1, y2[:, :, 0:1], y2[:, :, 1:2])
                        y0 = epi.tile([P, wb], f32, tag=f"{nm}0{sfx}", name=f"{nm}0{sfx}")
                        te.tensor_add(
                            y0.rearrange("p (t one) -> p t one", one=1),
                            y1, x[:, :, 4:5])
                        return y0

                    sw = pool_sum5("sw", E)
                    r = epi.tile([P, wb], f32, tag=f"r{wb}", name=f"r{sfx}")
                    if raw:
                        nc.vector.reciprocal(r, sw)
                    else:
                        nlsw = epi.tile([P, wb], f32, tag=f"nlsw{wb}", name=f"nlsw{sfx}")
                        nc.scalar.activation(out=nlsw, in_=sw, func=AF.Ln)
                        nc.scalar.activation(out=r, in_=nlsw, func=AF.Exp,
                                             scale=-1.0)
                    U = epi.tile([P, wb, TOPK], bf16, tag=f"U{wb}", name=f"U{sfx}")
                    te.tensor_mul(U, E, reb_sb[:, 0:wb])
                    UZ = epi.tile([P, wb, TOPK], bf16, tag=f"UZ{wb}", name=f"UZ{sfx}")
                    te.tensor_mul(UZ, U, zf_sb[:, sl])
                    s1 = pool_sum5("s1", UZ)
                    s2 = pool_sum5("s2", U)
                    t1 = epi.tile([P, wb], f32, tag=f"t1{wb}", name=f"t1{sfx}")
                    te.tensor_mul(t1, lse, s2)
                    t1b = epi.tile([P, wb], f32, tag=f"t1b{wb}", name=f"t1b{sfx}")
                    te.tensor_sub(t1b, t1, s1)
                    if raw:
                        te.tensor_mul(acc[:, col:col + wb], t1b, r)
                        return
                    nc.gpsimd.tensor_mul(t1b, t1b, r)
                    pp = t1b
                    w2 = wb // 2
                    lv = 0
                    while w2 >= 1:
                        nxt = epi.tile([P, w2], f32, tag=f"pp{lv}{sfx}", name=f"pp{lv}{sfx}")
                        nc.gpsimd.tensor_add(nxt, pp[:, 0:w2], pp[:, w2:2 * w2])
                        pp = nxt
                        w2 //= 2
                        lv += 1
                    nc.gpsimd.tensor_copy(acc[:, col:col + 1], pp)

                if g in (3, 7, 11):
                    bi = g // 4
                    emit_epi(slice(bi * TB, (bi + 1) * TB), bi * TB, TB,
                             f"b{bi}", raw=True)
                elif g == NG - 2:
                    emit_epi(slice(192, 240), 192, 48, "h", raw=True)
                elif g == NG - 1:
                    emit_tree(0, 8, nc.vector, "a")
                    emit_epi(slice(240, 248), 240, 8, "h2a", raw=True)
                    emit_tree(8, 8, nc.vector, "b")
                    emit_epi(slice(248, 256), 248, 8, "h2b", raw=True)

            nc.sync.dma_start(out=out_d[:, :], in_=acc)

    # tail drain: keep only the out-DMA completion wait (see v2)
    out_sems = set()
    for blk in nc.m.functions[0].blocks:
        for inst in blk.instructions:
            si = getattr(inst, "sync_info", None)
            if si is None or type(inst).__name__ != "InstDMACopy":
                continue
            try:
                if inst.outs[0].memref == "acc":
                    out_sems |= {u.ant_name for u in si.on_update}
            except Exception:
                pass
    assert out_sems, "could not locate the output DMA's completion sem"
    for blk in nc.m.functions[0].blocks:
        for inst in blk.instructions:
            si = getattr(inst, "sync_info", None)
            if si is None or type(inst).__name__ != "InstDrain":
                continue
            ge = [w for w in si.on_wait if w.wait_mode == "sem-ge-imm"]
            if len(ge) >= 2:
                keep = [w for w in ge if w.ant_name in out_sems]
                assert keep, f"tail drain has no out-DMA wait: {ge}"
                si.on_wait = keep[:1]
    import bass_rust
    bass_rust.generate_event_semaphores(nc)
    return nc


def _get_nc(Kp=None, goff=None, segc=None):
    global _NC, _NC_KEY
    if Kp is None:
        assert _NC is not None, "kernel not built yet"
        return _NC
    key = (tuple(Kp), segc)
    if _NC is None or _NC_KEY != key:
        _NC = _build_nc(Kp, goff, segc)
        _NC_KEY = key
    return _NC


def _morton(a, b, bits=8):
    qa = np.clip(((a + 110.0) / 220.0 * (1 << bits)).astype(np.int64),
                 0, (1 << bits) - 1)
    qb = np.clip(((b + 110.0) / 220.0 * (1 << bits)).astype(np.int64),
                 0, (1 << bits) - 1)
    m = np.zeros_like(qa)
    for i in range(bits):
        m |= ((qa >> i) & 1) << (2 * i) | ((qb >> i) & 1) << (2 * i + 1)
    return m


def make_in_maps(Zbar, Y, rebalance, gamut):
    Zbar = np.asarray(Zbar, dtype=np.float32)
    Y = np.asarray(Y, dtype=np.float32)
    rebalance = np.asarray(rebalance, dtype=np.float32)
    gamut = np.asarray(gamut, dtype=np.float32)
    gx, gy = gamut[:, 0], gamut[:, 1]
    gn = gx * gx + gy * gy

    rebt = np.ascontiguousarray(
        np.broadcast_to(np.tile(rebalance[:TOPK], TB)[None, :], (P, TB * TOPK))
    ).astype(ml_dtypes.bfloat16)

    # --- per-core: Morton-sort pixels, build per-tile candidate sets ---
    per_core = []
    for c in range(NCORES):
        sl = slice(c * B_PER, (c + 1) * B_PER)
        a = Y[sl, 1].reshape(PIX)
        b = Y[sl, 2].reshape(PIX)
        order = np.argsort(_morton(a, b), kind="stable")
        a, b = a[order], b[order]
        cands = []
        for t in range(NT):
            ts = slice(t * P, (t + 1) * P)
            ta, tb = a[ts], b[ts]
            lo = np.array([ta.min(), tb.min()])
            hi = np.array([ta.max(), tb.max()])
            ctr = (lo + hi) / 2
            halfdiag = float(np.linalg.norm(hi - lo)) / 2
            dc = np.sqrt(((gamut - ctr) ** 2).sum(1))
            d5c = np.partition(dc, TOPK - 1)[TOPK - 1]
            dx = np.maximum(np.maximum(lo[0] - gx, gx - hi[0]), 0)
            dy = np.maximum(np.maximum(lo[1] - gy, gy - hi[1]), 0)
            dbb = np.sqrt(dx * dx + dy * dy)
            cand = np.where(dbb <= d5c + halfdiag + 1e-3)[0]
            cands.append(cand)
        per_core.append((order, a, b, cands))

    # shared padded widths across cores (one kernel build): per tile index,
    # max over cores, padded to a multiple of 8, min 16
    Kp = []
    for t in range(NT):
        k = max(len(per_core[c][3][t]) for c in range(NCORES))
        Kp.append(max((k + 7) // 8 * 8, 16))
    goff = []
    off = 0
    seg_of = lambda t: 0 if t < SEG_START[1] else (1 if t < SEG_START[2] else 2)
    for t in range(NT):
        if t in SEG_START:
            off = 0
        goff.append(off)
        off += Kp[t]
    garea = [sum(Kp[SEG_START[s]:SEG_START[s] + SEG_TILES[s]])
             for s in range(NSEG)]
    segc = max(garea) + max(SEG_TILES) * P

    in_maps = []
    for c in range(NCORES):
        order, a, b, cands = per_core[c]
        sl = slice(c * B_PER, (c + 1) * B_PER)
        zb = Zbar[sl].reshape(PIX, Q)[order]
        zb16 = np.ascontiguousarray(zb.astype(ml_dtypes.bfloat16))
        zf = np.ascontiguousarray(
            zb[:, 0:TOPK].reshape(NT, P, TOPK).transpose(1, 0, 2).reshape(P, NT * TOPK)
        ).astype(ml_dtypes.bfloat16)
        # bf16 hi/lo split: m = ah*gxh + ah*gxl + al*gxh + (same for b/gy)
        #                     + 1*(-nh) + 1*(-nl)   (8-row contraction)
        def hl(x):
            h = x.astype(ml_dtypes.bfloat16)
            l = (x - h.astype(np.float32)).astype(ml_dtypes.bfloat16)
            return h, l
        abx = np.zeros((72, segc), ml_dtypes.bfloat16)
        GAR = max(garea)
        gxh, gxl = hl(2.0 * gx)
        gyh, gyl = hl(2.0 * gy)
        gnh, gnl = hl(-gn)
        ah, al = hl(a)
        bh, bl = hl(b)
        for s in range(NSEG):
            r = 32 * s
            npx = SEG_TILES[s] * P
            px = slice(SEG_START[s] * P, SEG_START[s] * P + npx)
            p0 = GAR
            for ri, row in enumerate((ah[px], ah[px], al[px],
                                      bh[px], bh[px], bl[px])):
                abx[r + ri, p0:p0 + npx] = row
            abx[r + 6, p0:p0 + npx] = 1.0
            abx[r + 7, p0:p0 + npx] = 1.0
            for t in range(SEG_START[s], SEG_START[s] + SEG_TILES[s]):
                cd = cands[t]
                o = goff[t]
                n = len(cd)
                for ri, row in enumerate((gxh[cd], gxl[cd], gxh[cd],
                                          gyh[cd], gyl[cd], gyh[cd],
                                          gnh[cd], gnl[cd])):
                    abx[r + ri, o:o + n] = row
                abx[r + 6, o + n:o + Kp[t]] = -3.0e38
                abx[r + 7, o + n:o + Kp[t]] = 0.0
        in_maps.append({"zbar": zb16, "abx": abx, "zf": zf, "rebt": rebt})
    return in_maps, Kp, goff, segc


def kernel(Zbar, Y, rebalance, gamut):
    in_maps, Kp, goff, segc = make_in_maps(Zbar, Y, rebalance, gamut)
    nc = _get_nc(Kp, goff, segc)
    res = run_bass_kernel_spmd(nc, in_maps, list(range(NCORES)))
    total = sum(float(r["acc"].sum(dtype=np.float64)) for r in res.results)
    return np.float32(total / (B * H * W))


# revision 53
# speedup vs baseline: 1.0389x; 1.0385x over previous
"""Trainium2 Bass kernel for the colorization loss — v5.

Math (validated to rel-err ~1e-6):
  m(q)  = 2*a*gx_q + 2*b*gy_q - |g_q|^2        # = (a^2+b^2) - d^2(q)
  top-5 largest m == 5 nearest bins (sorted);  e_k = exp((m_k - m_0)/50)
  loss  = mean( (lse * sum_k(reb_k e_k) - sum_k(reb_k e_k zbar_k)) / sum_k e_k )
  with lse = log(sum_q exp(zbar_q)).

v5 = v2 + per-tile gamut candidate pruning:
  Pixels are HOST-SORTED in Morton order over (a,b), so each 128-pixel
  tile occupies a small patch of ab-space.  For each tile the host
  computes (from the tiny gamut + the tile's bbox, rigorously via the
  d5 Lipschitz bound) the set of bins that can possibly be in any of
  its pixels' top-5 — mean ~40 of 313 (verified exact on all pixels).
  Each tile's matmul then only produces its own candidate columns and
  DVE's top-8 scan shrinks from 313 to K_t columns (451ns -> ~170ns
  avg), moving the bottleneck to ACT's exp stream (~74us floor).
  Per-pixel loss partials are order-independent, so the sort needs no
  undo.  The exp/sum(exp) path is unchanged except the add-tree is
  rebalanced: level 1 on Pool, the rest + final reduce on the
  now-lighter DVE.  3 abx segments (bases 0/32/64), each laid out
  [per-tile candidate blocks | a;b;1 pixel columns].
"""

import numpy as np
import ml_dtypes

import concourse.bass as bass
import concourse.tile as tile
from concourse import mybir
from concourse.bass_utils import run_bass_kernel_spmd

# Problem shape (hardcoded: nn_ColorizationLoss, B,H,W,Q = 16,128,128,313)
B, H, W, Q = 16, 128, 128, 313
NCORES = 8
B_PER = B // NCORES            # 2 images per core
PIX = B_PER * H * W            # 32768 pixels per core
P = 128                        # SBUF partitions / pixels per tile
NT = PIX // P                  # 256 tiles per core
GT = 16                        # tiles per zbar group
NG = NT // GT                  # 16 groups
TB = 64                        # tiles per epilogue batch
NB = NT // TB                  # 8 batches
TOPK = 5
INV50 = 1.0 / 50.0             # 1/(2*sigma^2), sigma=5
QPAD = 320                     # es padded to 320 cols for a clean add-tree
ESBUF = 5                      # rotating es buffers

NSEG = 3                       # abx partition segments (bases 0/32/64)
SEG_TILES = (86, 85, 85)
SEG_START = (0, 86, 171)

f32 = mybir.dt.float32
bf16 = mybir.dt.bfloat16
f32r = mybir.dt.float32r
AF = mybir.ActivationFunctionType
AX = mybir.AxisListType
ALU = mybir.AluOpType

_NC = None
_NC_KEY = None


def _build_nc(Kp, goff, segc):
    """Kp[t]: padded candidate count per tile; goff[t]: column offset of
    tile t's gamut block within its segment; segc: columns per abx row."""
    nc = bass.Bass()
    zbar_d = nc.dram_tensor("zbar", [PIX, Q], bf16, kind="ExternalInput")
    abx_d = nc.dram_tensor("abx", [72, segc], bf16, kind="ExternalInput")
    zf_d = nc.dram_tensor("zf", [P, NT * TOPK], bf16, kind="ExternalInput")
    reb_d = nc.dram_tensor("rebt", [P, TB * TOPK], bf16, kind="ExternalInput")
    out_d = nc.dram_tensor("acc", [P, NT], f32, kind="ExternalOutput")

    # pixel-column start per segment (gamut blocks lead the segment row)
    gtot = [goff[SEG_START[s]] + sum(Kp[t] for t in range(SEG_START[s],
            SEG_START[s] + SEG_TILES[s])) - goff[SEG_START[s]]
            for s in range(NSEG)]
    garea = [sum(Kp[SEG_START[s]:SEG_START[s] + SEG_TILES[s]])
             for s in range(NSEG)]
    pix0 = [garea[s] for s in range(NSEG)]

    zbar_g = zbar_d[:, :].rearrange("(g j p) q -> g p j q", j=GT, p=P)

    with tile.TileContext(nc) as tc:
        with (
            tc.tile_pool(name="singles", bufs=1) as singles,
            tc.tile_pool(name="zg", bufs=3) as zgp,
            tc.tile_pool(name="tree", bufs=2) as trp,
            tc.tile_pool(name="epi", bufs=1) as epi_big,
            tc.tile_pool(name="epi2", bufs=2) as epi_small,
            tc.tile_pool(name="ps", bufs=4, space="PSUM") as psp,
            tc.tile_pool(name="psb", bufs=1, space="PSUM") as psbp,
        ):
            abx_sb = singles.tile([8 + 32 * (NSEG - 1), segc], bf16)
            # one [67, cols] dma moves the same column range of all three
            # segments in parallel partition lanes (the DMA model charges
            # max bytes PER PARTITION, so narrow 3-row transfers are 3x
            # the cost of this stacked layout).  gamut blocks ride the
            # gpsimd ring; pixel columns stream on the scalar ring.
            GAR = max(garea)
            # es pads zeroed FIRST (decode before the dma SEQ-holds; the
            # DVE-run early sum trees need them)
            es_bufs = []
            for i in range(ESBUF):
                e = singles.tile([P, GT, QPAD], bf16, name=f"es{i}")
                nc.gpsimd.memset(e[:, :, Q:QPAD], 0.0)
                es_bufs.append(e)
            nc.gpsimd.dma_start(out=abx_sb[0:72, 0:GAR], in_=abx_d[:, 0:GAR])

            zf_sb = singles.tile([P, NT, TOPK], bf16)
            nc.gpsimd.dma_start(
                out=zf_sb, in_=zf_d[:, :].rearrange("p (t k) -> p t k", k=TOPK)
            )
            reb_sb = singles.tile([P, TB, TOPK], bf16)
            nc.gpsimd.dma_start(
                out=reb_sb, in_=reb_d[:, :].rearrange("p (t k) -> p t k", k=TOPK)
            )
            acc = singles.tile([P, NT], f32)

            Sf = singles.tile([P, NT], f32)          # sum_q exp(zbar)
            Wf = singles.tile([P, NT, 8], f32)       # top-8 of m

            def issue_zg(g):
                zgt = es_bufs[g % ESBUF]
                nc.sync.dma_start(out=zgt[:, :, 0:Q], in_=zbar_g[g])

            # zg0 arrives in two pieces so exp can start ~1.5us earlier
            nc.sync.dma_start(out=es_bufs[0][:, 0:4, 0:Q], in_=zbar_g[0][:, 0:4, :])
            nc.sync.dma_start(out=es_bufs[0][:, 4:GT, 0:Q], in_=zbar_g[0][:, 4:GT, :])
            # exp(0) decodes BEFORE the pixel pieces: engine ops release
            # the ACT SEQ at dispatch, so the dma SEQ-holds below hide
            # under exp(0)'s engine time instead of delaying the spine
            e0 = es_bufs[0]
            nc.scalar.activation(out=e0[:, 0:4, 0:Q], in_=e0[:, 0:4, 0:Q],
                                 func=AF.Exp)
            nc.scalar.activation(out=e0[:, 4:GT, 0:Q], in_=e0[:, 4:GT, 0:Q],
                                 func=AF.Exp)
            # pixel columns in 3 pieces on the scalar ring, column order
            PXP = [GAR, GAR + 32 * P, GAR + 64 * P, segc]
            for pi in range(3):
                nc.scalar.dma_start(out=abx_sb[0:72, PXP[pi]:PXP[pi + 1]],
                                    in_=abx_d[:, PXP[pi]:PXP[pi + 1]])
            issue_zg(1)

            for g in range(NG):
                if g + 2 < NG:
                    issue_zg(g + 2)

                # PE: 16 matmuls on per-tile candidate blocks; DVE max8
                for j in range(GT):
                    t = g * GT + j
                    seg = 0 if t < SEG_START[1] else (1 if t < SEG_START[2] else 2)
                    so = 32 * seg
                    col = max(garea) + (t - SEG_START[seg]) * P
                    k = Kp[t]
                    # fresh psum slot where the matmul's abx piece may be
                    # the last-arriving wait (single-wait LDWEIGHTS rule)
                    if t in (0, 16, SEG_START[1], SEG_START[2]):
                        ps = psbp.tile([P, QPAD], f32, tag=f"psb{t}", name=f"psb{t}")
                    else:
                        ps = psp.tile([P, QPAD], f32, tag="ps")
                    nc.tensor.matmul(
                        ps[:, 0:k],
                        abx_sb[so:so + 8, col:col + P],
                        abx_sb[so:so + 8, goff[t]:goff[t] + k],
                        start=True,
                        stop=True,
                    )
                    nc.vector.max(out=Wf[:, t, :], in_=ps[:, 0:k])

                # ACT: one batched exp per group, IN PLACE (g0 split so
                # the first exp starts on the first dma piece; last group
                # split 24/8 so the tail tree+epilogue overlap exp)
                es = es_bufs[g % ESBUF]
                ze = es[:, :, 0:Q]
                if g == 0:
                    pass  # exp(0) emitted before the pixel pieces above
                elif g == NG - 1:
                    nc.scalar.activation(out=es[:, 0:8, 0:Q], in_=es[:, 0:8, 0:Q],
                                         func=AF.Exp)
                    nc.scalar.activation(out=es[:, 8:GT, 0:Q], in_=es[:, 8:GT, 0:Q],
                                         func=AF.Exp)
                else:
                    nc.scalar.activation(out=ze, in_=ze, func=AF.Exp)

                # sum tree: level 1 on Pool, rest + reduce on DVE
                def emit_tree(j0, w, l1_eng, sfx2=""):
                    # full-width tiles (shared tags) sliced to w: the half
                    # trees of the last group reuse the main tree buffers
                    sl2 = slice(g * GT + j0, g * GT + j0 + w)
                    t160f = trp.tile([P, GT, 160], bf16, tag="t160",
                                     name=f"t160_{g}{sfx2}")
                    t160 = t160f[:, 0:w, :]
                    l1_eng.tensor_add(t160, es[:, j0:j0 + w, 0:160],
                                      es[:, j0:j0 + w, 160:320])
                    t80f = trp.tile([P, GT, 80], bf16, tag="t80",
                                    name=f"t80_{g}{sfx2}")
                    t80 = t80f[:, 0:w, :]
                    l1_eng.tensor_add(t80, t160[:, :, 0:80],
                                      t160[:, :, 80:160])
                    t40f = trp.tile([P, GT, 40], bf16, tag="t40",
                                    name=f"t40_{g}{sfx2}")
                    t40 = t40f[:, 0:w, :]
                    l1_eng.tensor_add(t40, t80[:, :, 0:40], t80[:, :, 40:80])
                    nc.vector.tensor_reduce(
                        Sf[:, sl2].rearrange("p (t one) -> p t one", one=1),
                        t40, AX.X, ALU.add,
                    )

                if g < NG - 1:
                    emit_tree(0, GT, nc.gpsimd)

                # ---- epilogue (from v2): batches 0-6 of 32 tiles; the last
                # 32 tiles split 16/15/1 so the tail chain stays short ----
                def emit_epi(sl, col, wb, sfx, raw=False):
                    epi = epi_small if wb <= 8 else epi_big
                    sub_eng = nc.vector if raw else nc.gpsimd
                    te = nc.gpsimd
                    Xb = epi.tile([P, wb, TOPK], bf16, tag=f"X{wb}", name=f"X{sfx}")
                    sub_eng.tensor_sub(
                        Xb, Wf[:, sl, 0:TOPK],
                        Wf[:, sl, 0:1].broadcast_to([P, wb, TOPK]),
                    )
                    E = epi.tile([P, wb, TOPK], bf16, tag=f"E{wb}", name=f"E{sfx}")
                    nc.scalar.activation(out=E, in_=Xb, func=AF.Exp,
                                         scale=INV50)
                    lse = epi.tile([P, wb], f32, tag=f"lse{wb}", name=f"lse{sfx}")
                    nc.scalar.activation(out=lse, in_=Sf[:, sl], func=AF.Ln)

                    def pool_sum5(nm, x):
                        y2 = epi.tile([P, wb, 2], f32, tag=f"{nm}2{sfx}", name=f"{nm}2{sfx}")
                        te.tensor_add(y2, x[:, :, 0:2], x[:, :, 2:4])
                        y1 = epi.tile([P, wb, 1], f32, tag=f"{nm}1{sfx}", name=f"{nm}1{sfx}")
                        te.tensor_add(y1, y2[:, :, 0:1], y2[:, :, 1:2])
                        y0 = epi.tile([P, wb], f32, tag=f"{nm}0{sfx}", name=f"{nm}0{sfx}")
                        te.tensor_add(
                            y0.rearrange("p (t one) -> p t one", one=1),
                            y1, x[:, :, 4:5])
                        return y0

                    sw = pool_sum5("sw", E)
                    r = epi.tile([P, wb], f32, tag=f"r{wb}", name=f"r{sfx}")
                    if raw:
                        nc.vector.reciprocal(r, sw)
                    else:
                        nlsw = epi.tile([P, wb], f32, tag=f"nlsw{wb}", name=f"nlsw{sfx}")
                        nc.scalar.activation(out=nlsw, in_=sw, func=AF.Ln)
                        nc.scalar.activation(out=r, in_=nlsw, func=AF.Exp,
                                             scale=-1.0)
                    U = epi.tile([P, wb, TOPK], bf16, tag=f"U{wb}", name=f"U{sfx}")
                    te.tensor_mul(U, E, reb_sb[:, 0:wb])
                    UZ = epi.tile([P, wb, TOPK], bf16, tag=f"UZ{wb}", name=f"UZ{sfx}")
                    te.tensor_mul(UZ, U, zf_sb[:, sl])
                    s1 = pool_sum5("s1", UZ)
                    s2 = pool_sum5("s2", U)
                    t1 = epi.tile([P, wb], f32, tag=f"t1{wb}", name=f"t1{sfx}")
                    te.tensor_mul(t1, lse, s2)
                    t1b = epi.tile([P, wb], f32, tag=f"t1b{wb}", name=f"t1b{sfx}")
                    te.tensor_sub(t1b, t1, s1)
                    if raw:
                        te.tensor_mul(acc[:, col:col + wb], t1b, r)
                        return
                    nc.gpsimd.tensor_mul(t1b, t1b, r)
                    pp = t1b
                    w2 = wb // 2
                    lv = 0
                    while w2 >= 1:
                        nxt = epi.tile([P, w2], f32, tag=f"pp{lv}{sfx}", name=f"pp{lv}{sfx}")
                        nc.gpsimd.tensor_add(nxt, pp[:, 0:w2], pp[:, w2:2 * w2])
                        pp = nxt
                        w2 //= 2
                        lv += 1
                    nc.gpsimd.tensor_copy(acc[:, col:col + 1], pp)

                if g in (3, 7, 11):
                    bi = g // 4
                    emit_epi(slice(bi * TB, (bi + 1) * TB), bi * TB, TB,
                             f"b{bi}", raw=True)
                elif g == NG - 2:
                    emit_epi(slice(192, 240), 192, 48, "h", raw=True)
                elif g == NG - 1:
                    emit_tree(0, 8, nc.vector, "a")
                    emit_epi(slice(240, 248), 240, 8, "h2a", raw=True)
                    emit_tree(8, 8, nc.vector, "b")
                    emit_epi(slice(248, 256), 248, 8, "h2b", raw=True)

            nc.sync.dma_start(out=out_d[:, :], in_=acc)

    # tail drain: keep only the out-DMA completion wait (see v2)
    out_sems = set()
    for blk in nc.m.functions[0].blocks:
        for inst in blk.instructions:
            si = getattr(inst, "sync_info", None)
            if si is None or type(inst).__name__ != "InstDMACopy":
                continue
            try:
                if inst.outs[0].memref == "acc":
                    out_sems |= {u.ant_name for u in si.on_update}
            except Exception:
                pass
    assert out_sems, "could not locate the output DMA's completion sem"
    for blk in nc.m.functions[0].blocks:
        for inst in blk.instructions:
            si = getattr(inst, "sync_info", None)
            if si is None or type(inst).__name__ != "InstDrain":
                continue
            ge = [w for w in si.on_wait if w.wait_mode == "sem-ge-imm"]
            if len(ge) >= 2:
                keep = [w for w in ge if w.ant_name in out_sems]
                assert keep, f"tail drain has no out-DMA wait: {ge}"
                si.on_wait = keep[:1]
    import bass_rust
    bass_rust.generate_event_semaphores(nc)
    return nc


def _get_nc(Kp=None, goff=None, segc=None):
    global _NC, _NC_KEY
    if Kp is None:
        assert _NC is not None, "kernel not built yet"
        return _NC
    key = (tuple(Kp), segc)
    if _NC is None or _NC_KEY != key:
        _NC = _build_nc(Kp, goff, segc)
        _NC_KEY = key
    return _NC


def _morton(a, b, bits=8):
    qa = np.clip(((a + 110.0) / 220.0 * (1 << bits)).astype(np.int64),
                 0, (1 << bits) - 1)
    qb = np.clip(((b + 110.0) / 220.0 * (1 << bits)).astype(np.int64),
                 0, (1 << bits) - 1)
    m = np.zeros_like(qa)
    for i in range(bits):
        m |= ((qa >> i) & 1) << (2 * i) | ((qb >> i) & 1) << (2 * i + 1)
    return m


def make_in_maps(Zbar, Y, rebalance, gamut):
    Zbar = np.asarray(Zbar, dtype=np.float32)
    Y = np.asarray(Y, dtype=np.float32)
    rebalance = np.asarray(rebalance, dtype=np.float32)
    gamut = np.asarray(gamut, dtype=np.float32)
    gx, gy = gamut[:, 0], gamut[:, 1]
    gn = gx * gx + gy * gy

    rebt = np.ascontiguousarray(
        np.broadcast_to(np.tile(rebalance[:TOPK], TB)[None, :], (P, TB * TOPK))
    ).astype(ml_dtypes.bfloat16)

    # --- per-core: Morton-sort pixels, build per-tile candidate sets ---
    per_core = []
    for c in range(NCORES):
        sl = slice(c * B_PER, (c + 1) * B_PER)
        a = Y[sl, 1].reshape(PIX)
        b = Y[sl, 2].reshape(PIX)
        order = np.argsort(_morton(a, b), kind="stable")
        a, b = a[order], b[order]
        cands = []
        for t in range(NT):
            ts = slice(t * P, (t + 1) * P)
            ta, tb = a[ts], b[ts]
            lo = np.array([ta.min(), tb.min()])
            hi = np.array([ta.max(), tb.max()])
            ctr = (lo + hi) / 2
            halfdiag = float(np.linalg.norm(hi - lo)) / 2
            dc = np.sqrt(((gamut - ctr) ** 2).sum(1))
            d5c = np.partition(dc, TOPK - 1)[TOPK - 1]
            dx = np.maximum(np.maximum(lo[0] - gx, gx - hi[0]), 0)
            dy = np.maximum(np.maximum(lo[1] - gy, gy - hi[1]), 0)
            dbb = np.sqrt(dx * dx + dy * dy)
            cand = np.where(dbb <= d5c + halfdiag + 1e-3)[0]
            cands.append(cand)
        per_core.append((order, a, b, cands))

    # shared padded widths across cores (one kernel build): per tile index,
    # max over cores, padded to a multiple of 8, min 16
    Kp = []
    for t in range(NT):
        k = max(len(per_core[c][3][t]) for c in range(NCORES))
        Kp.append(max((k + 7) // 8 * 8, 16))
    goff = []
    off = 0
    seg_of = lambda t: 0 if t < SEG_START[1] else (1 if t < SEG_START[2] else 2)
    for t in range(NT):
        if t in SEG_START:
            off = 0
        goff.append(off)
        off += Kp[t]
    garea = [sum(Kp[SEG_START[s]:SEG_START[s] + SEG_TILES[s]])
             for s in range(NSEG)]
    segc = max(garea) + max(SEG_TILES) * P

    in_maps = []
    for c in range(NCORES):
        order, a, b, cands = per_core[c]
        sl = slice(c * B_PER, (c + 1) * B_PER)
        zb = Zbar[sl].reshape(PIX, Q)[order]
        zb16 = np.ascontiguousarray(zb.astype(ml_dtypes.bfloat16))
        zf = np.ascontiguousarray(
            zb[:, 0:TOPK].reshape(NT, P, TOPK).transpose(1, 0, 2).reshape(P, NT * TOPK)
        ).astype(ml_dtypes.bfloat16)
        # bf16 hi/lo split: m = ah*gxh + ah*gxl + al*gxh + (same for b/gy)
        #                     + 1*(-nh) + 1*(-nl)   (8-row contraction)
        def hl(x):
            h = x.astype(ml_dtypes.bfloat16)
            l = (x - h.astype(np.float32)).astype(ml_dtypes.bfloat16)
            return h, l
        abx = np.zeros((72, segc), ml_dtypes.bfloat16)
        GAR = max(garea)
        gxh, gxl = hl(2.0 * gx)
        gyh, gyl = hl(2.0 * gy)
        gnh, gnl = hl(-gn)
        ah, al = hl(a)
        bh, bl = hl(b)
        for s in range(NSEG):
            r = 32 * s
            npx = SEG_TILES[s] * P
            px = slice(SEG_START[s] * P, SEG_START[s] * P + npx)
            p0 = GAR
            for ri, row in enumerate((ah[px], ah[px], al[px],
                                      bh[px], bh[px], bl[px])):
                abx[r + ri, p0:p0 + npx] = row
            abx[r + 6, p0:p0 + npx] = 1.0
            abx[r + 7, p0:p0 + npx] = 1.0
            for t in range(SEG_START[s], SEG_START[s] + SEG_TILES[s]):
                cd = cands[t]
                o = goff[t]
                n = len(cd)
                for ri, row in enumerate((gxh[cd], gxl[cd], gxh[cd],
                                          gyh[cd], gyl[cd], gyh[cd],
                                          gnh[cd], gnl[cd])):
                    abx[r + ri, o:o + n] = row
                abx[r + 6, o + n:o + Kp[t]] = -3.0e38
                abx[r + 7, o + n:o + Kp[t]] = 0.0
        in_maps.append({"zbar": zb16, "abx": abx, "zf": zf, "rebt": rebt})
    return in_maps, Kp, goff, segc


def kernel(Zbar, Y, rebalance, gamut):
    in_maps, Kp, goff, segc = make_in_maps(Zbar, Y, rebalance, gamut)
    nc = _get_nc(Kp, goff, segc)
    res = run_bass_kernel_spmd(nc, in_maps, list(range(NCORES)))
    total = sum(float(r["acc"].sum(dtype=np.float64)) for r in res.results)
    return np.float32(total / (B * H * W))
